# revision 2
# baseline (speedup 1.0000x reference)
import numpy as np

_CACHE = {}

N_CORES = 8
TOK = 16384
TOK_PER = TOK // N_CORES  # 2048 tokens per core
DIM = 2048
NE = 64
TOPK = 8
KC = 128            # contraction chunk (partition dim)
NK = DIM // KC      # 16 chunks
NT = 512            # token tile = one f32 PSUM bank
NJ = TOK_PER // NT  # 4 token tiles


def _build():
    import concourse.bass as bass
    import concourse.tile as tile
    from concourse import bacc, mybir

    nc = bacc.Bacc(
        "TRN2",
        target_bir_lowering=False,
        debug=False,
        enable_asserts=False,
        num_devices=N_CORES,
    )
    xT = nc.dram_tensor("xT", (DIM, TOK_PER), mybir.dt.float32, kind="ExternalInput").ap()
    wT = nc.dram_tensor("WT", (DIM, NE), mybir.dt.float32, kind="ExternalInput").ap()
    out = nc.dram_tensor("logitsT", (NE, TOK_PER), mybir.dt.float32, kind="ExternalOutput").ap()

    with tile.TileContext(nc) as tc:
        with (
            tc.tile_pool(name="xpool", bufs=NK) as xpool,
            tc.tile_pool(name="wpool", bufs=NK) as wpool,
            tc.tile_pool(name="opool", bufs=NJ) as opool,
            tc.tile_pool(name="psum", bufs=NJ, space=bass.MemorySpace.PSUM) as psum,
        ):
            xts, wts = [], []
            for k in range(NK):
                xt = xpool.tile([KC, TOK_PER], mybir.dt.float32)
                nc.gpsimd.dma_start(xt[:], xT[k * KC:(k + 1) * KC, :])
                wt = wpool.tile([KC, NE], mybir.dt.float32)
                nc.gpsimd.dma_start(wt[:], wT[k * KC:(k + 1) * KC, :])
                xts.append(xt)
                wts.append(wt)
            for j in range(NJ):
                acc = psum.tile([NE, NT], mybir.dt.float32)
                for k in range(NK):
                    nc.tensor.matmul(
                        acc[:],
                        wts[k][:],
                        xts[k][:, j * NT:(j + 1) * NT],
                        start=(k == 0),
                        stop=(k == NK - 1),
                    )
                ot = opool.tile([NE, NT], mybir.dt.float32)
                nc.vector.tensor_copy(ot[:], acc[:])
                nc.gpsimd.dma_start(out[:, j * NT:(j + 1) * NT], ot[:])
    nc.compile()
    return nc


def _make_in_maps(x, W):
    x = np.asarray(x, dtype=np.float32)
    W = np.asarray(W, dtype=np.float32)
    WT = np.ascontiguousarray(W.T)
    in_maps = []
    for i in range(N_CORES):
        xs = x[i * TOK_PER:(i + 1) * TOK_PER]
        in_maps.append({"xT": np.ascontiguousarray(xs.T), "WT": WT})
    return in_maps


def kernel(x, W):
    from concourse import bass_utils

    if "nc" not in _CACHE:
        _CACHE["nc"] = _build()
    nc = _CACHE["nc"]

    in_maps = _make_in_maps(x, W)
    res = bass_utils.run_bass_kernel_spmd(nc, in_maps, list(range(N_CORES)))
    logits = np.concatenate(
        [np.asarray(r["logitsT"]).T for r in res.results], axis=0
    ).astype(np.float32)

    m = logits.max(axis=-1, keepdims=True)
    e = np.exp(logits - m)
    scores = e / e.sum(axis=-1, keepdims=True)
    idx = np.argsort(-scores, axis=-1, kind="stable")[:, :TOPK].astype(np.int32)
    w = np.take_along_axis(scores, idx, axis=-1).astype(np.float32)
    return w, idx



# revision 5
# speedup vs baseline: 1.7553x; 1.7553x over previous
import numpy as np

_CACHE = {}

N_CORES = 8
TOK = 16384
TOK_PER = TOK // N_CORES  # 2048 tokens per core
DIM = 2048
NE = 64
TOPK = 8
KC = 128            # contraction chunk (partition dim)
NK = DIM // KC      # 16 chunks
NT = 512            # token tile = one f32 PSUM bank
NJ = TOK_PER // NT  # 4 token tiles

# x-slab DMA granularity: groups of K-chunks per dma_start.
# Bigger slabs amortize DMA overhead; a small final slab shrinks the
# compute tail after the last byte lands.
SLABS = [(0, 2), (2, 4), (4, 6), (6, 8), (8, 10), (10, 12), (12, 14), (14, 15), (15, 16)]


def _build():
    import concourse.bass as bass
    import concourse.tile as tile
    from concourse import bacc, mybir

    nc = bacc.Bacc(
        "TRN2",
        target_bir_lowering=False,
        debug=False,
        enable_asserts=False,
        num_devices=N_CORES,
    )
    # xpk[p, k*TOK_PER + t] = x[t, k*KC + p]  (fp16, host-packed)
    xpk = nc.dram_tensor("xpk", (KC, NK * TOK_PER), mybir.dt.float16, kind="ExternalInput").ap()
    # wpk[p, k*NE + e] = W[e, k*KC + p]  (fp16, host-packed)
    wpk = nc.dram_tensor("wpk", (KC, NK * NE), mybir.dt.float16, kind="ExternalInput").ap()
    out = nc.dram_tensor("logitsT", (NE, TOK_PER), mybir.dt.float32, kind="ExternalOutput").ap()

    with tile.TileContext(nc) as tc:
        with (
            tc.tile_pool(name="xpool", bufs=len(SLABS)) as xpool,
            tc.tile_pool(name="wpool", bufs=1) as wpool,
            tc.tile_pool(name="opool", bufs=NJ) as opool,
            tc.tile_pool(name="psum", bufs=NJ, space=bass.MemorySpace.PSUM) as psum,
        ):
            wsb = wpool.tile([KC, NK * NE], mybir.dt.float16)
            nc.scalar.dma_start(wsb[:], wpk[:, :])

            slabs = []
            for (k0, k1) in SLABS:
                xt = xpool.tile([KC, (k1 - k0) * TOK_PER], mybir.dt.float16)
                nc.sync.dma_start(xt[:], xpk[:, k0 * TOK_PER:k1 * TOK_PER])
                slabs.append(xt)

            accs = [
                psum.tile([NE, NT], mybir.dt.float32, tag=f"acc{j}", bufs=1, name=f"acc{j}")
                for j in range(NJ)
            ]
            for xt, (k0, k1) in zip(slabs, SLABS):
                for k in range(k0, k1):
                    lhs = wsb[:, k * NE:(k + 1) * NE]
                    off = (k - k0) * TOK_PER
                    for j in range(NJ):
                        nc.tensor.matmul(
                            accs[j][:],
                            lhs,
                            xt[:, off + j * NT:off + (j + 1) * NT],
                            start=(k == 0),
                            stop=(k == NK - 1),
                        )
            for j in range(NJ):
                ot = opool.tile([NE, NT], mybir.dt.float32)
                nc.vector.tensor_copy(ot[:], accs[j][:])
                nc.sync.dma_start(out[:, j * NT:(j + 1) * NT], ot[:])
    nc.compile()
    return nc


def _pack_x(xs):
    # xs: [TOK_PER, DIM] fp32 -> [KC, NK*TOK_PER] fp16 packed chunks
    xT = xs.T.astype(np.float16)  # [DIM, TOK_PER]
    return np.ascontiguousarray(
        xT.reshape(NK, KC, TOK_PER).transpose(1, 0, 2).reshape(KC, NK * TOK_PER)
    )


def _make_in_maps(x, W):
    x = np.asarray(x, dtype=np.float32)
    W = np.asarray(W, dtype=np.float32)
    WT = W.T.astype(np.float16)  # [DIM, NE]
    wpk = np.ascontiguousarray(
        WT.reshape(NK, KC, NE).transpose(1, 0, 2).reshape(KC, NK * NE)
    )
    in_maps = []
    for i in range(N_CORES):
        xs = x[i * TOK_PER:(i + 1) * TOK_PER]
        in_maps.append({"xpk": _pack_x(xs), "wpk": wpk})
    return in_maps


def kernel(x, W):
    from concourse import bass_utils

    if "nc" not in _CACHE:
        _CACHE["nc"] = _build()
    nc = _CACHE["nc"]

    in_maps = _make_in_maps(x, W)
    res = bass_utils.run_bass_kernel_spmd(nc, in_maps, list(range(N_CORES)))
    logits = np.concatenate(
        [np.asarray(r["logitsT"]).T for r in res.results], axis=0
    ).astype(np.float32)

    m = logits.max(axis=-1, keepdims=True)
    e = np.exp(logits - m)
    scores = e / e.sum(axis=-1, keepdims=True)
    idx = np.argsort(-scores, axis=-1, kind="stable")[:, :TOPK].astype(np.int32)
    w = np.take_along_axis(scores, idx, axis=-1).astype(np.float32)
    return w, idx


# revision 6
# speedup vs baseline: 1.9943x; 1.1362x over previous
import numpy as np

_CACHE = {}

N_CORES = 8
TOK = 16384
TOK_PER = TOK // N_CORES  # 2048 tokens per core
DIM = 2048
NE = 64
TOPK = 8
KC = 128            # contraction chunk (partition dim)
NK = DIM // KC      # 16 chunks
NT = 512            # token tile = one f32 PSUM bank
NJ = TOK_PER // NT  # 4 token tiles
WCOLS = NK * NE     # 1024 weight columns (all chunks, packed)

# x-slab DMA granularity: chunk pairs. Slab 0 additionally carries the
# packed weights so the first matmul isn't gated on a separate slow
# small-descriptor W transfer.
NSLAB = NK // 2  # 8 slabs of 2 chunks


def _build():
    import concourse.bass as bass
    import concourse.tile as tile
    from concourse import bacc, mybir

    nc = bacc.Bacc(
        "TRN2",
        target_bir_lowering=False,
        debug=False,
        enable_asserts=False,
        num_devices=N_CORES,
    )
    # xpk[:, 0:WCOLS] = packed weights: wpk[p, k*NE + e] = W[e, k*KC + p]
    # xpk[:, WCOLS + k*TOK_PER + t] = x[t, k*KC + p]
    xpk = nc.dram_tensor(
        "xpk", (KC, WCOLS + NK * TOK_PER), mybir.dt.float16, kind="ExternalInput"
    ).ap()
    # packed output: rows 0:64 = even-chunk partial logits, rows 64:128 = odd
    # (host sums the halves); columns = tokens of the core shard.
    out = nc.dram_tensor("opk", (KC, TOK_PER), mybir.dt.float16, kind="ExternalOutput").ap()

    with tile.TileContext(nc) as tc:
        with (
            tc.tile_pool(name="xpool", bufs=NSLAB) as xpool,
            tc.tile_pool(name="opool", bufs=1) as opool,
            tc.tile_pool(name="psum", bufs=NJ, space=bass.MemorySpace.PSUM) as psum,
        ):
            slabs = []
            for s in range(NSLAB):
                w = WCOLS if s == 0 else 0
                xt = xpool.tile([KC, w + 2 * TOK_PER], mybir.dt.float16)
                lo = 0 if s == 0 else WCOLS + 2 * s * TOK_PER
                nc.sync.dma_start(xt[:], xpk[:, lo:lo + w + 2 * TOK_PER])
                slabs.append(xt)
            wsb = slabs[0]  # weights live in slab 0's first WCOLS columns

            accs = [
                psum.tile([KC, NT], mybir.dt.float32, tag=f"acc{j}", bufs=1, name=f"acc{j}")
                for j in range(NJ)
            ]
            o128 = opool.tile([KC, TOK_PER], mybir.dt.float16)

            for s in range(NSLAB):
                xt = slabs[s]
                ka, kb = 2 * s, 2 * s + 1
                lw_a = wsb[:, ka * NE:(ka + 1) * NE]
                lw_b = wsb[:, kb * NE:(kb + 1) * NE]
                xoff = WCOLS if s == 0 else 0
                for j in range(NJ):
                    # even chunk -> PE col group 0 -> PSUM partitions 0:64
                    nc.tensor.matmul(
                        accs[j][0:NE, :],
                        lw_a,
                        xt[:, xoff + j * NT:xoff + (j + 1) * NT],
                        start=(ka == 0),
                        stop=(ka == NK - 2),
                    )
                    # odd chunk -> PE col group 1 -> PSUM partitions 64:128
                    nc.tensor.matmul(
                        accs[j][NE:2 * NE, :],
                        lw_b,
                        xt[:, xoff + TOK_PER + j * NT:xoff + TOK_PER + (j + 1) * NT],
                        start=(kb == 1),
                        stop=(kb == NK - 1),
                    )
            for j in range(NJ):
                nc.vector.tensor_copy(o128[:, j * NT:(j + 1) * NT], accs[j][:])
            nc.scalar.dma_start(out[:, :], o128[:])
    nc.compile()
    return nc


def _make_in_maps(x, W):
    x = np.asarray(x, dtype=np.float32)
    W = np.asarray(W, dtype=np.float32)
    WT = W.T.astype(np.float16)  # [DIM, NE]
    wpk = WT.reshape(NK, KC, NE).transpose(1, 0, 2).reshape(KC, WCOLS)
    in_maps = []
    for i in range(N_CORES):
        xs = x[i * TOK_PER:(i + 1) * TOK_PER]
        xT = xs.T.astype(np.float16)  # [DIM, TOK_PER]
        xp = xT.reshape(NK, KC, TOK_PER).transpose(1, 0, 2).reshape(KC, NK * TOK_PER)
        in_maps.append({"xpk": np.ascontiguousarray(np.concatenate([wpk, xp], axis=1))})
    return in_maps


def kernel(x, W):
    from concourse import bass_utils

    if "nc" not in _CACHE:
        _CACHE["nc"] = _build()
    nc = _CACHE["nc"]

    in_maps = _make_in_maps(x, W)
    res = bass_utils.run_bass_kernel_spmd(nc, in_maps, list(range(N_CORES)))
    parts = []
    for r in res.results:
        o = np.asarray(r["opk"], dtype=np.float32)  # [128, TOK_PER]
        parts.append((o[:NE, :] + o[NE:, :]).T)     # [TOK_PER, NE]
    logits = np.concatenate(parts, axis=0)

    m = logits.max(axis=-1, keepdims=True)
    e = np.exp(logits - m)
    scores = e / e.sum(axis=-1, keepdims=True)
    idx = np.argsort(-scores, axis=-1, kind="stable")[:, :TOPK].astype(np.int32)
    w = np.take_along_axis(scores, idx, axis=-1).astype(np.float32)
    return w, idx
